# revision 28
# baseline (speedup 1.0000x reference)
"""Multi-head causal attention (B=4,T=2048,E=1024,H=16,D=64) on 8 TRN2 cores.

Sharding: core c -> batch b=c//2, heads h0=(c%2)*8 (tensor-parallel over head
halves within each batch). Each core computes its 8 heads' attention and a
partial output projection (row-split Wp); host sums the two partials per
batch (+bias).

v6, selective precision + engine balance:
  - j=0 (queries 0..511, the largest-|out| rows): fully bf16 pipeline.
  - j>=1: fp8e4 DoubleRow for Q/K/V projections and scores; PV fp8 (DR over
    chunk pairs off-diagonal, single-chunk on the diagonal). Softmax
    averaging over >=512 keys washes out fp8 quantization noise.
  - V path scaled x32 uniformly (ones column = 32) so fp8 stays in range;
    the o/z ratio cancels the scale exactly.
  - exp: ACT for most tiles; j=3 off-diagonal pairs use a Schraudolph
    bit-exp split DVE (affine->int32) + Pool (bitcast copy) to offload the
    ACT bottleneck. Causal masks for j>=1 run on Pool, j0 on DVE.
  - PE warmup matmuls mature the clock ramp during the initial DMA wait.
"""
import sys
import numpy as np

sys.path.insert(0, "/opt/trn_rl_repo")

import ml_dtypes
import concourse.bass as bass
import concourse.bacc as bacc
import concourse.mybir as mybir
from concourse import tile
from concourse.bass_utils import run_bass_kernel_spmd

B, T, E, H, D = 4, 2048, 1024, 16, 64
HL = H // 2      # 8 local heads per core
NP = HL // 2     # 4 head pairs
NJ = T // 512    # 4 tq tiles
NCK = T // 128   # 16 t chunks
NE = E // 128    # 8 e chunks (bf16)
NE2 = E // 256   # 4 e pair-chunks (fp8 DoubleRow)
BF16 = mybir.dt.bfloat16
F32 = mybir.dt.float32
FP8 = mybir.dt.float8e4
I32 = mybir.dt.int32
bfnp = ml_dtypes.bfloat16
fp8np = ml_dtypes.float8_e4m3
DR = mybir.MatmulPerfMode.DoubleRow
MUL = mybir.AluOpType.mult
ADD = mybir.AluOpType.add

WSCALE = 32.0
VSCALE = 32.0
EXPSCALE8 = 0.125 / (WSCALE * WSCALE)
EXPSCALE = 0.125
LOG2E = 1.4426950408889634
SCH_A = (1 << 23) * LOG2E
SCH_B = float((127 << 23) - 366393)

_CACHE = {}


def _build():
    nc = bacc.Bacc("TRN2", target_bir_lowering=False)
    xT = nc.declare_dram_parameter("xT", [E, T], BF16, isOutput=False)
    xT8 = nc.declare_dram_parameter("xT8", [E, T], FP8, isOutput=False)
    wq = nc.declare_dram_parameter("wq", [E, HL * D], BF16, isOutput=False)
    wk = nc.declare_dram_parameter("wk", [E, HL * D], BF16, isOutput=False)
    wv = nc.declare_dram_parameter("wv", [E, HL * D], BF16, isOutput=False)
    w8q = nc.declare_dram_parameter("w8q", [E, HL * D], FP8, isOutput=False)
    w8k = nc.declare_dram_parameter("w8k", [E, HL * D], FP8, isOutput=False)
    w8v = nc.declare_dram_parameter("w8v", [E, HL * D], FP8, isOutput=False)
    wpT = nc.declare_dram_parameter("wpT", [HL * D, E], BF16, isOutput=False)
    maskb = nc.declare_dram_parameter("maskb", [128, 128], FP8, isOutput=False)
    maskb16 = nc.declare_dram_parameter("maskb16", [128, 128], BF16, isOutput=False)
    out = nc.declare_dram_parameter("out", [T, E], F32, isOutput=True)

    Exp = mybir.ActivationFunctionType.Exp

    with tile.TileContext(nc) as tc:
        with (
            tc.tile_pool(name="persist", bufs=1) as pp,
            tc.tile_pool(name="qtmp", bufs=3) as qtmp,
            tc.tile_pool(name="expp", bufs=2) as expp,
            tc.tile_pool(name="expp8", bufs=4) as expp8,
            tc.tile_pool(name="i32p", bufs=2) as i32p,
            tc.tile_pool(name="zpool", bufs=2) as zpool,
            tc.tile_pool(name="outp", bufs=3) as outp,
            tc.tile_pool(name="mm", bufs=2, space=bass.MemorySpace.PSUM) as ps_mm,
            tc.tile_pool(name="sT", bufs=2, space=bass.MemorySpace.PSUM) as ps_sT,
            tc.tile_pool(name="oT", bufs=2, space=bass.MemorySpace.PSUM) as ps_oT,
        ):
            xT_all = pp.tile([128, NE, T], BF16, tag="xTa", name="xTa")
            x8_all = pp.tile([128, NE2, 2, T], FP8, tag="x8a", name="x8a")
            wq_all = pp.tile([128, NE, HL * D], BF16, tag="bqa", name="bqa")
            wk_all = pp.tile([128, NE, HL * D], BF16, tag="bka", name="bka")
            wv_all = pp.tile([128, NE, HL * D], BF16, tag="wva", name="wva")
            w8q_all = pp.tile([128, NE2, 2, 512], FP8, tag="wqa", name="wqa")
            w8k_all = pp.tile([128, NE2, 2, 512], FP8, tag="wka", name="wka")
            w8v_all = pp.tile([128, NE2, 2, 512], FP8, tag="wvb", name="wvb")
            wpT_all = pp.tile([128, NP, E], BF16, tag="wpa", name="wpa")
            mask_sb = pp.tile([128, 128], FP8, tag="mkb", name="mkb")
            mask16_sb = pp.tile([128, 128], BF16, tag="mk16", name="mk16")
            # j0 V (bf16, x32): chunks 0..3
            v_sb = [pp.tile([128, HL * 65], BF16, tag=f"v{i}", name=f"v{i}")
                    for i in range(4)]
            # fp8 V (x32): [128, HL, 2, 128] per chunk pair - per-head [2,128]
            # block so the DoubleRow Ldweights pair stride is 128; v at cols
            # 0:64, ones (=32) at col 64
            v8_sb = [pp.tile([128, HL, 2, 128], FP8, tag=f"v8{i}", name=f"v8{i}")
                     for i in range(8)]
            qb_sb = [pp.tile([128, 512], BF16, tag=f"qb{p}", name=f"qb{p}") for p in range(NP)]
            kb_sb = [pp.tile([128, 512], BF16, tag=f"kb{p}", name=f"kb{p}") for p in range(NP)]
            q8_sb = [pp.tile([128, 2, T], FP8, tag=f"q8{p}", name=f"q8{p}") for p in range(NP)]
            k8_sb = [pp.tile([128, 2, T], FP8, tag=f"k8{p}", name=f"k8{p}") for p in range(NP)]
            oT_sb = [[pp.tile([128, 512], BF16, tag=f"o{p}_{j}", name=f"o{p}_{j}") for j in range(NJ)]
                     for p in range(NP)]

            # PE warmup: matmuls on a memset tile so the p-state ramp matures
            # while the first DMAs are in flight
            wu = pp.tile([128, 256], BF16, tag="wu", name="wu")
            nc.gpsimd.memset(wu[:], 0.0)
            for _ in range(14):
                wu_ps = ps_mm.tile([128, 256], F32, tag="mm", name="wups")
                nc.tensor.matmul(wu_ps[:], wu[:, 0:128], wu[:], start=True,
                                 stop=True)

            # ---- DMA in ----
            def chunked(dram, c):
                return dram[:].rearrange("(c p) t -> p c t", c=c)

            def paired(dram, col0, col1):
                return dram[:, col0:col1].rearrange(
                    "(c i p) t -> p c i t", c=NE2, i=2)

            nc.scalar.dma_start(wq_all[:], chunked(wq, NE))
            nc.scalar.dma_start(wk_all[:], chunked(wk, NE))
            nc.sync.dma_start(xT_all[:, :, 0:512],
                              xT[:, 0:512].rearrange("(c p) t -> p c t", c=NE))
            nc.sync.dma_start(wv_all[:], chunked(wv, NE))
            nc.sync.dma_start(mask_sb[:], maskb[:])
            nc.sync.dma_start(mask16_sb[:], maskb16[:])
            nc.sync.dma_start(w8q_all[:], paired(w8q, 0, 512))
            nc.sync.dma_start(w8k_all[:], paired(w8k, 0, 512))
            nc.sync.dma_start(w8v_all[:], paired(w8v, 0, 512))
            for w in range(1, 4):
                nc.sync.dma_start(x8_all[:, :, :, w * 512:(w + 1) * 512],
                                  paired(xT8, w * 512, (w + 1) * 512))
                nc.sync.dma_start(
                    xT_all[:, :, w * 512:(w + 1) * 512],
                    xT[:, w * 512:(w + 1) * 512].rearrange("(c p) t -> p c t", c=NE))
            nc.sync.dma_start(wpT_all[:], chunked(wpT, NP))

            # ---- emission helpers ----
            def fold_dmas(dst, p, jcol, tmp):
                for hh in range(2):
                    for half in range(2):
                        nc.sync.dma_start(
                            dst[p][64 * hh:64 * hh + 32, half, jcol:jcol + 512],
                            tmp[hh * 64 + half * 32:hh * 64 + half * 32 + 32, :])

            def j0_proj(p):
                for wsb, bdst, is_k in ((wq_all, qb_sb, False), (wk_all, kb_sb, True)):
                    ps = ps_mm.tile([128, 512], F32, tag="mm", name="mmq")
                    for c in range(NE):
                        nc.tensor.matmul(ps[:], wsb[:, c, p * 128:(p + 1) * 128],
                                         xT_all[:, c, 0:512],
                                         start=(c == 0), stop=(c == NE - 1))
                    nc.vector.tensor_copy(bdst[p][:], ps[:])
                    if is_k:
                        tmp = qtmp.tile([128, 512], FP8, tag="qt", name="qt")
                        nc.vector.tensor_scalar_mul(tmp[:], ps[:], WSCALE)
                        fold_dmas(k8_sb, p, 0, tmp)

            def jn_proj(j, p, w8_sb, dst):
                ps = ps_mm.tile([128, 512], F32, tag="mm", name="mmq")
                for c2 in range(NE2):
                    nc.tensor.matmul(
                        ps[:], w8_sb[:, c2, :, p * 128:(p + 1) * 128],
                        x8_all[:, c2, :, j * 512:(j + 1) * 512],
                        start=(c2 == 0), stop=(c2 == NE2 - 1), perf_mode=DR)
                tmp = qtmp.tile([128, 512], FP8, tag="qt", name="qt")
                nc.vector.tensor_copy(tmp[:], ps[:])
                fold_dmas(dst, p, j * 512, tmp)

            def v_proj(i):
                # chunks 0..3: bf16 projection (j0 exactness); 4..15: fp8 DR
                ps = ps_mm.tile([128, HL * D], F32, tag="mm", name="mmv")
                if i < 4:
                    for c in range(NE):
                        nc.tensor.matmul(ps[:], xT_all[:, c, i * 128:(i + 1) * 128],
                                         wv_all[:, c], start=(c == 0),
                                         stop=(c == NE - 1))
                else:
                    for c2 in range(NE2):
                        nc.tensor.matmul(
                            ps[:], x8_all[:, c2, :, i * 128:(i + 1) * 128],
                            w8v_all[:, c2], start=(c2 == 0),
                            stop=(c2 == NE2 - 1), perf_mode=DR)
                cp, ci = i // 2, i % 2
                if ci == 0:
                    nc.gpsimd.memset(v8_sb[cp][:], VSCALE)
                ps3 = ps[:].rearrange("p (h d) -> p h d", d=64)
                if i < 4:
                    # bf16 psum is x1: scale x32 into both v8 and v_sb
                    nc.vector.tensor_scalar(v8_sb[cp][:, :, ci, 0:64], ps3,
                                            VSCALE, None, MUL)
                    v3 = v_sb[i][:].rearrange("p (h d) -> p h d", d=65)
                    nc.gpsimd.memset(v_sb[i][:], VSCALE)
                    nc.vector.tensor_scalar(v3[:, :, 0:64], ps3, VSCALE, None, MUL)
                else:
                    # fp8 psum already x32 (w8v pre-scaled)
                    nc.vector.tensor_copy(v8_sb[cp][:, :, ci, 0:64], ps3)

            def attn(j, h):
                p, hh = h // 2, h % 2
                rb = 64 * hh
                oT_ps = ps_oT.tile([65, 512], F32, tag="oT", name="oTps")
                nblk = 4 * (j + 1)
                npair = nblk // 2
                for cp in range(npair):
                    offdiag = j > 0 and cp < 2 * j
                    cw = []
                    for ci in range(2):
                        c = 2 * cp + ci
                        r = max(0, (c - 4 * j) * 128)
                        cw.append((c, r, 512 - r))
                    sT = ps_sT.tile([128, 2, 512], F32, tag="sT", name="sTps")
                    for ci, (c, r, w) in enumerate(cw):
                        if j == 0:
                            nc.tensor.matmul(
                                sT[:, ci, 0:w],
                                kb_sb[p][rb:rb + 64, c * 128:(c + 1) * 128],
                                qb_sb[p][rb:rb + 64, r:512],
                                start=True, stop=True)
                        else:
                            nc.tensor.matmul(
                                sT[:, ci, 0:w],
                                k8_sb[p][rb:rb + 32, :, c * 128:(c + 1) * 128],
                                q8_sb[p][rb:rb + 32, :, j * 512 + r:(j + 1) * 512],
                                start=True, stop=True, perf_mode=DR)
                    if offdiag:
                        et8 = expp8.tile([128, 2, 512], FP8, tag="e8", name="e8")
                        if j == 3 and cp < 3:
                            # Schraudolph bit-exp: DVE affine->int32, Pool
                            # bitcast copy -> fp8 (offloads the ACT engine)
                            it = i32p.tile([128, 2, 512], I32, tag="i32", name="i32")
                            nc.vector.tensor_scalar(
                                it[:], sT[:], EXPSCALE8 * SCH_A, SCH_B, MUL, ADD)
                            nc.gpsimd.tensor_copy(et8[:], it[:].bitcast(F32))
                        else:
                            nc.scalar.activation(et8[:], sT[:], Exp,
                                                 scale=EXPSCALE8)
                        nc.tensor.matmul(
                            oT_ps[:, 0:512],
                            v8_sb[cp][:, h, :, 0:65],
                            et8[:],
                            start=(cp == 0), stop=False,
                            perf_mode=DR, skip_group_check=True)
                        continue
                    # diagonal pair
                    if j == 0:
                        et = expp.tile([128, 2, 512], BF16, tag="et", name="et")
                    else:
                        et = expp8.tile([128, 2, 512], FP8, tag="e8", name="e8")
                    scale = EXPSCALE if j == 0 else EXPSCALE8
                    w0 = cw[0][2]
                    if w0 == 512:       # widths 512, 384
                        sflat = sT[:].rearrange("p a b -> p (a b)")
                        eflat = et[:].rearrange("p a b -> p (a b)")
                        nc.scalar.activation(eflat[:, 0:896], sflat[:, 0:896],
                                             Exp, scale=scale)
                    else:               # widths 256, 128
                        nc.scalar.activation(et[:, :, 0:128], sT[:, :, 0:128],
                                             Exp, scale=scale)
                        nc.scalar.activation(et[:, 0, 128:256], sT[:, 0, 128:256],
                                             Exp, scale=scale)
                    for ci, (c, r, w) in enumerate(cw):
                        if j == 0:
                            nc.vector.tensor_mul(et[:, ci, 0:128],
                                                 et[:, ci, 0:128], mask16_sb[:])
                        else:
                            nc.gpsimd.tensor_mul(et[:, ci, 0:128],
                                                 et[:, ci, 0:128], mask_sb[:])
                    for ci, (c, r, w) in enumerate(cw):
                        if j == 0:
                            v3 = v_sb[c][:].rearrange("p (h d) -> p h d", d=65)
                            lhsT = v3[:, h, :]
                        else:
                            lhsT = v8_sb[c // 2][:, h, c % 2, 0:65]
                        nc.tensor.matmul(oT_ps[:, r:512], lhsT,
                                         et[:, ci, 0:w],
                                         start=(c == 0), stop=(c == nblk - 1),
                                         skip_group_check=True)
                zi = zpool.tile([1, 512], F32, tag="zi", name="zi")
                nc.vector.reciprocal(zi[:], oT_ps[64:65, :])
                zb = zpool.tile([64, 512], F32, tag="zb", name="zb")
                nc.gpsimd.partition_broadcast(zb[:], zi[:])
                nc.vector.tensor_mul(oT_sb[p][j][64 * hh:64 * hh + 64, :],
                                     oT_ps[0:64, :], zb[:])

            def out_proj(j, t):
                ob = outp.tile([128, E], F32, tag="ob", name="ob")
                for n in range(2):
                    ps = ps_mm.tile([128, 512], F32, tag="mm", name="mmo")
                    for p in range(NP):
                        nc.tensor.matmul(
                            ps[:],
                            oT_sb[p][j][:, (t % 4) * 128:(t % 4 + 1) * 128],
                            wpT_all[:, p, n * 512:(n + 1) * 512],
                            start=(p == 0), stop=(p == NP - 1))
                    nc.vector.tensor_copy(ob[:, n * 512:(n + 1) * 512], ps[:])
                    nc.sync.dma_start(
                        out[t * 128:(t + 1) * 128, n * 512:(n + 1) * 512],
                        ob[:, n * 512:(n + 1) * 512])

            # ---- emission schedule ----
            j0_proj(0)
            for i in range(4):
                v_proj(i)

            fillers = {
                0: ([("j0p", 1), ("pq", 1, 0), ("pk", 1, 0), ("j0p", 2),
                     ("j0p", 3), ("pq", 1, 1), ("pk", 1, 1), ("v", 4),
                     ("pq", 1, 2), ("pk", 1, 2), ("v", 5),
                     ("pq", 1, 3), ("pk", 1, 3), ("v", 6), ("v", 7)]),
                1: ([("pq", 2, 0), ("pk", 2, 0), ("v", 8), ("o", 0, 0),
                     ("pq", 2, 1), ("pk", 2, 1), ("v", 9), ("o", 0, 1),
                     ("pq", 2, 2), ("pk", 2, 2), ("v", 10), ("o", 0, 2),
                     ("pq", 2, 3), ("pk", 2, 3), ("v", 11), ("o", 0, 3)]),
                2: ([("pq", 3, 0), ("pk", 3, 0), ("v", 12), ("o", 1, 0),
                     ("pq", 3, 1), ("pk", 3, 1), ("v", 13), ("o", 1, 1),
                     ("pq", 3, 2), ("pk", 3, 2), ("v", 14), ("o", 1, 2),
                     ("pq", 3, 3), ("pk", 3, 3), ("v", 15), ("o", 1, 3)]),
                3: ([("o", 2, 0), ("o", 2, 1), ("o", 2, 2), ("o", 2, 3)]),
            }

            def run_filler(f):
                if f[0] == "v":
                    v_proj(f[1])
                elif f[0] == "j0p":
                    j0_proj(f[1])
                elif f[0] == "pq":
                    jn_proj(f[1], f[2], w8q_all, q8_sb)
                elif f[0] == "pk":
                    jn_proj(f[1], f[2], w8k_all, k8_sb)
                elif f[0] == "o":
                    out_proj(f[1], f[1] * 4 + f[2])

            for j in range(NJ):
                q = list(fillers[j])
                per = (len(q) + HL - 1) // HL
                for h in range(HL):
                    attn(j, h)
                    for _ in range(per):
                        if q:
                            run_filler(q.pop(0))
                while q:
                    run_filler(q.pop(0))
            for t in range(12, 16):
                out_proj(3, t)

    nc.compile()
    return nc


def _mask_np(dt):
    f = np.arange(128)[None, :]
    p = np.arange(128)[:, None]
    return (f >= p).astype(dt)


def kernel(x, Wq, Wk, Wv, Wp, bp):
    x = np.asarray(x, dtype=np.float32)
    Wq = np.asarray(Wq, dtype=np.float32)
    Wk = np.asarray(Wk, dtype=np.float32)
    Wv = np.asarray(Wv, dtype=np.float32)
    Wp = np.asarray(Wp, dtype=np.float32)
    bp = np.asarray(bp, dtype=np.float32)

    if "nc" not in _CACHE:
        _CACHE["nc"] = _build()
    nc = _CACHE["nc"]

    WpT = np.ascontiguousarray(Wp.T).astype(bfnp)
    xTs = [np.ascontiguousarray(x[b].T) for b in range(B)]

    def wslice(W, h0, scale, dt):
        return np.ascontiguousarray(
            (W[h0:h0 + HL] * scale).transpose(1, 0, 2).reshape(E, HL * D)).astype(dt)

    in_maps = []
    for c in range(8):
        b, hhf = c // 2, c % 2
        h0 = hhf * HL
        in_maps.append({
            "xT": xTs[b].astype(bfnp),
            "xT8": xTs[b].astype(fp8np),
            "wq": wslice(Wq, h0, 1.0, bfnp),
            "wk": wslice(Wk, h0, 1.0, bfnp),
            "wv": wslice(Wv, h0, 1.0, bfnp),
            "w8q": wslice(Wq, h0, WSCALE, fp8np),
            "w8k": wslice(Wk, h0, WSCALE, fp8np),
            "w8v": wslice(Wv, h0, VSCALE, fp8np),
            "wpT": np.ascontiguousarray(WpT[h0 * D:(h0 + HL) * D, :]),
            "maskb": _mask_np(fp8np),
            "maskb16": _mask_np(bfnp),
        })

    res = run_bass_kernel_spmd(nc, in_maps, list(range(8)))
    parts = [np.asarray(res.results[c]["out"], dtype=np.float32) for c in range(8)]
    out = np.stack([parts[2 * b] + parts[2 * b + 1] for b in range(B)], axis=0)
    return (out + bp[None, None, :]).astype(np.float32)


# revision 31
# speedup vs baseline: 1.1059x; 1.1059x over previous
"""Multi-head causal attention (B=4,T=2048,E=1024,H=16,D=64) on 8 TRN2 cores.

Sharding: core c -> batch b=c//2, heads h0=(c%2)*8 (tensor-parallel over head
halves within each batch). Each core computes its 8 heads' attention and a
partial output projection (row-split Wp); host sums the two partials per
batch (+bias).

v6, selective precision + engine balance:
  - j=0 (queries 0..511, the largest-|out| rows): fully bf16 pipeline.
  - j>=1: fp8e4 DoubleRow for Q/K/V projections and scores; PV fp8 (DR over
    chunk pairs off-diagonal, single-chunk on the diagonal). Softmax
    averaging over >=512 keys washes out fp8 quantization noise.
  - V path scaled x32 uniformly (ones column = 32) so fp8 stays in range;
    the o/z ratio cancels the scale exactly.
  - exp: ACT for most tiles; j=3 off-diagonal pairs use a Schraudolph
    bit-exp split DVE (affine->int32) + Pool (bitcast copy) to offload the
    ACT bottleneck. Causal masks for j>=1 run on Pool, j0 on DVE.
  - PE warmup matmuls mature the clock ramp during the initial DMA wait.
"""
import sys
import numpy as np

sys.path.insert(0, "/opt/trn_rl_repo")

import ml_dtypes
import concourse.bass as bass
import concourse.bacc as bacc
import concourse.mybir as mybir
from concourse import tile
from concourse.bass_utils import run_bass_kernel_spmd

B, T, E, H, D = 4, 2048, 1024, 16, 64
HL = H // 2      # 8 local heads per core
NP = HL // 2     # 4 head pairs
NJ = T // 512    # 4 tq tiles
NCK = T // 128   # 16 t chunks
NE = E // 128    # 8 e chunks (bf16)
NE2 = E // 256   # 4 e pair-chunks (fp8 DoubleRow)
BF16 = mybir.dt.bfloat16
F32 = mybir.dt.float32
FP8 = mybir.dt.float8e4
I32 = mybir.dt.int32
bfnp = ml_dtypes.bfloat16
fp8np = ml_dtypes.float8_e4m3
DR = mybir.MatmulPerfMode.DoubleRow
MUL = mybir.AluOpType.mult
ADD = mybir.AluOpType.add

WSCALE = 32.0
VSCALE = 32.0
EXPSCALE8 = 0.125 / (WSCALE * WSCALE)
EXPSCALE = 0.125
LOG2E = 1.4426950408889634
# fp8e4m3 byte-space exp: byte = round(arg*8*log2e + 56 + C); the fp8 decode
# of that byte is exp(arg) to ~3% rms (piecewise-log byte layout)
SCH8_A = 8.0 * LOG2E
SCH8_B = 56.0 - 0.45

_CACHE = {}


def _build():
    nc = bacc.Bacc("TRN2", target_bir_lowering=False)
    xT = nc.declare_dram_parameter("xT", [E, T], BF16, isOutput=False)
    xT8 = nc.declare_dram_parameter("xT8", [E, T], FP8, isOutput=False)
    wq = nc.declare_dram_parameter("wq", [E, HL * D], BF16, isOutput=False)
    wk = nc.declare_dram_parameter("wk", [E, HL * D], BF16, isOutput=False)
    wv = nc.declare_dram_parameter("wv", [E, HL * D], BF16, isOutput=False)
    w8q = nc.declare_dram_parameter("w8q", [E, HL * D], FP8, isOutput=False)
    w8k = nc.declare_dram_parameter("w8k", [E, HL * D], FP8, isOutput=False)
    w8v = nc.declare_dram_parameter("w8v", [E, HL * D], FP8, isOutput=False)
    wpT = nc.declare_dram_parameter("wpT", [HL * D, E], BF16, isOutput=False)
    maskb = nc.declare_dram_parameter("maskb", [128, 128], FP8, isOutput=False)
    maskb16 = nc.declare_dram_parameter("maskb16", [128, 128], BF16, isOutput=False)
    out = nc.declare_dram_parameter("out", [T, E], F32, isOutput=True)

    Exp = mybir.ActivationFunctionType.Exp

    with tile.TileContext(nc) as tc:
        with (
            tc.tile_pool(name="persist", bufs=1) as pp,
            tc.tile_pool(name="qtmp", bufs=3) as qtmp,
            tc.tile_pool(name="expp", bufs=2) as expp,
            tc.tile_pool(name="expp8", bufs=4) as expp8,
            tc.tile_pool(name="i8p", bufs=3) as i8p,
            tc.tile_pool(name="zpool", bufs=2) as zpool,
            tc.tile_pool(name="outp", bufs=3) as outp,
            tc.tile_pool(name="mm", bufs=2, space=bass.MemorySpace.PSUM) as ps_mm,
            tc.tile_pool(name="sT", bufs=2, space=bass.MemorySpace.PSUM) as ps_sT,
            tc.tile_pool(name="oT", bufs=2, space=bass.MemorySpace.PSUM) as ps_oT,
        ):
            xT_all = pp.tile([128, NE, T], BF16, tag="xTa", name="xTa")
            x8_all = pp.tile([128, NE2, 2, T], FP8, tag="x8a", name="x8a")
            wq_all = pp.tile([128, NE, HL * D], BF16, tag="bqa", name="bqa")
            wk_all = pp.tile([128, NE, HL * D], BF16, tag="bka", name="bka")
            wv_all = pp.tile([128, NE, HL * D], BF16, tag="wva", name="wva")
            w8q_all = pp.tile([128, NE2, 2, 512], FP8, tag="wqa", name="wqa")
            w8k_all = pp.tile([128, NE2, 2, 512], FP8, tag="wka", name="wka")
            w8v_all = pp.tile([128, NE2, 2, 512], FP8, tag="wvb", name="wvb")
            wpT_all = pp.tile([128, NP, E], BF16, tag="wpa", name="wpa")
            mask_sb = pp.tile([128, 128], FP8, tag="mkb", name="mkb")
            mask16_sb = pp.tile([128, 128], BF16, tag="mk16", name="mk16")
            # j0 V (bf16, x32): chunks 0..3
            v_sb = [pp.tile([128, HL * 65], BF16, tag=f"v{i}", name=f"v{i}")
                    for i in range(4)]
            # fp8 V (x32): [128, HL, 2, 128] per chunk pair - per-head [2,128]
            # block so the DoubleRow Ldweights pair stride is 128; v at cols
            # 0:64, ones (=32) at col 64
            v8_sb = [pp.tile([128, HL, 2, 128], FP8, tag=f"v8{i}", name=f"v8{i}")
                     for i in range(8)]
            qb_sb = [pp.tile([128, 512], BF16, tag=f"qb{p}", name=f"qb{p}") for p in range(NP)]
            kb_sb = [pp.tile([128, 512], BF16, tag=f"kb{p}", name=f"kb{p}") for p in range(NP)]
            q8_sb = [pp.tile([128, 2, T], FP8, tag=f"q8{p}", name=f"q8{p}") for p in range(NP)]
            k8_sb = [pp.tile([128, 2, T], FP8, tag=f"k8{p}", name=f"k8{p}") for p in range(NP)]
            oT_sb = [[pp.tile([128, 512], BF16, tag=f"o{p}_{j}", name=f"o{p}_{j}") for j in range(NJ)]
                     for p in range(NP)]

            # PE warmup: matmuls on a memset tile so the p-state ramp matures
            # while the first DMAs are in flight
            wu = pp.tile([128, 256], BF16, tag="wu", name="wu")
            nc.gpsimd.memset(wu[:], 0.0)
            for _ in range(14):
                wu_ps = ps_mm.tile([128, 256], F32, tag="mm", name="wups")
                nc.tensor.matmul(wu_ps[:], wu[:, 0:128], wu[:], start=True,
                                 stop=True)

            # ---- DMA in ----
            def chunked(dram, c):
                return dram[:].rearrange("(c p) t -> p c t", c=c)

            def paired(dram, col0, col1):
                return dram[:, col0:col1].rearrange(
                    "(c i p) t -> p c i t", c=NE2, i=2)

            nc.scalar.dma_start(wq_all[:], chunked(wq, NE))
            nc.scalar.dma_start(wk_all[:], chunked(wk, NE))
            nc.sync.dma_start(xT_all[:, :, 0:512],
                              xT[:, 0:512].rearrange("(c p) t -> p c t", c=NE))
            nc.sync.dma_start(wv_all[:], chunked(wv, NE))
            nc.sync.dma_start(mask_sb[:], maskb[:])
            nc.sync.dma_start(mask16_sb[:], maskb16[:])
            nc.sync.dma_start(w8q_all[:], paired(w8q, 0, 512))
            nc.sync.dma_start(w8k_all[:], paired(w8k, 0, 512))
            nc.sync.dma_start(w8v_all[:], paired(w8v, 0, 512))
            for w in range(1, 4):
                nc.sync.dma_start(x8_all[:, :, :, w * 512:(w + 1) * 512],
                                  paired(xT8, w * 512, (w + 1) * 512))
                nc.sync.dma_start(
                    xT_all[:, :, w * 512:(w + 1) * 512],
                    xT[:, w * 512:(w + 1) * 512].rearrange("(c p) t -> p c t", c=NE))
            nc.sync.dma_start(wpT_all[:], chunked(wpT, NP))

            # ---- emission helpers ----
            def fold_dmas(dst, p, jcol, tmp):
                for hh in range(2):
                    for half in range(2):
                        nc.sync.dma_start(
                            dst[p][64 * hh:64 * hh + 32, half, jcol:jcol + 512],
                            tmp[hh * 64 + half * 32:hh * 64 + half * 32 + 32, :])

            def j0_proj(p):
                for wsb, bdst, is_k in ((wq_all, qb_sb, False), (wk_all, kb_sb, True)):
                    ps = ps_mm.tile([128, 512], F32, tag="mm", name="mmq")
                    for c in range(NE):
                        nc.tensor.matmul(ps[:], wsb[:, c, p * 128:(p + 1) * 128],
                                         xT_all[:, c, 0:512],
                                         start=(c == 0), stop=(c == NE - 1))
                    nc.vector.tensor_copy(bdst[p][:], ps[:])
                    if is_k:
                        tmp = qtmp.tile([128, 512], FP8, tag="qt", name="qt")
                        nc.vector.tensor_scalar_mul(tmp[:], ps[:], WSCALE)
                        fold_dmas(k8_sb, p, 0, tmp)

            def jn_proj(j, p, w8_sb, dst):
                ps = ps_mm.tile([128, 512], F32, tag="mm", name="mmq")
                for c2 in range(NE2):
                    nc.tensor.matmul(
                        ps[:], w8_sb[:, c2, :, p * 128:(p + 1) * 128],
                        x8_all[:, c2, :, j * 512:(j + 1) * 512],
                        start=(c2 == 0), stop=(c2 == NE2 - 1), perf_mode=DR)
                tmp = qtmp.tile([128, 512], FP8, tag="qt", name="qt")
                nc.vector.tensor_copy(tmp[:], ps[:])
                fold_dmas(dst, p, j * 512, tmp)

            def v_proj(i):
                # chunks 0..3: bf16 projection (j0 exactness); 4..15: fp8 DR
                ps = ps_mm.tile([128, HL * D], F32, tag="mm", name="mmv")
                if i < 4:
                    for c in range(NE):
                        nc.tensor.matmul(ps[:], xT_all[:, c, i * 128:(i + 1) * 128],
                                         wv_all[:, c], start=(c == 0),
                                         stop=(c == NE - 1))
                else:
                    for c2 in range(NE2):
                        nc.tensor.matmul(
                            ps[:], x8_all[:, c2, :, i * 128:(i + 1) * 128],
                            w8v_all[:, c2], start=(c2 == 0),
                            stop=(c2 == NE2 - 1), perf_mode=DR)
                cp, ci = i // 2, i % 2
                if ci == 0:
                    nc.gpsimd.memset(v8_sb[cp][:], VSCALE)
                ps3 = ps[:].rearrange("p (h d) -> p h d", d=64)
                if i < 4:
                    # bf16 psum is x1: scale x32 into both v8 and v_sb
                    nc.vector.tensor_scalar(v8_sb[cp][:, :, ci, 0:64], ps3,
                                            VSCALE, None, MUL)
                    v3 = v_sb[i][:].rearrange("p (h d) -> p h d", d=65)
                    nc.gpsimd.memset(v_sb[i][:], VSCALE)
                    nc.vector.tensor_scalar(v3[:, :, 0:64], ps3, VSCALE, None, MUL)
                else:
                    # fp8 psum already x32 (w8v pre-scaled)
                    nc.vector.tensor_copy(v8_sb[cp][:, :, ci, 0:64], ps3)

            def attn(j, h):
                p, hh = h // 2, h % 2
                rb = 64 * hh
                oT_ps = ps_oT.tile([65, 512], F32, tag="oT", name="oTps")
                nblk = 4 * (j + 1)
                npair = nblk // 2
                for cp in range(npair):
                    offdiag = j > 0 and cp < 2 * j
                    cw = []
                    for ci in range(2):
                        c = 2 * cp + ci
                        r = max(0, (c - 4 * j) * 128)
                        cw.append((c, r, 512 - r))
                    sT = ps_sT.tile([128, 2, 512], F32, tag="sT", name="sTps")
                    for ci, (c, r, w) in enumerate(cw):
                        if j == 0:
                            nc.tensor.matmul(
                                sT[:, ci, 0:w],
                                kb_sb[p][rb:rb + 64, c * 128:(c + 1) * 128],
                                qb_sb[p][rb:rb + 64, r:512],
                                start=True, stop=True)
                        else:
                            nc.tensor.matmul(
                                sT[:, ci, 0:w],
                                k8_sb[p][rb:rb + 32, :, c * 128:(c + 1) * 128],
                                q8_sb[p][rb:rb + 32, :, j * 512 + r:(j + 1) * 512],
                                start=True, stop=True, perf_mode=DR)
                    if offdiag:
                        if j == 3 and cp % 2 == 0:
                            # byte-space exp on DVE: one affine into int8,
                            # bitcast to fp8 (offloads the ACT engine)
                            it = i8p.tile([128, 2, 512], mybir.dt.int8,
                                          tag="i8", name="i8")
                            nc.vector.tensor_scalar(
                                it[:], sT[:], EXPSCALE8 * SCH8_A, SCH8_B,
                                MUL, ADD)
                            et8 = it[:].bitcast(FP8)
                        else:
                            et8t = expp8.tile([128, 2, 512], FP8, tag="e8",
                                              name="e8")
                            nc.scalar.activation(et8t[:], sT[:], Exp,
                                                 scale=EXPSCALE8)
                            et8 = et8t[:]
                        nc.tensor.matmul(
                            oT_ps[:, 0:512],
                            v8_sb[cp][:, h, :, 0:65],
                            et8,
                            start=(cp == 0), stop=False,
                            perf_mode=DR, skip_group_check=True)
                        continue
                    # diagonal pair
                    if j == 0:
                        et = expp.tile([128, 2, 512], BF16, tag="et", name="et")
                    else:
                        et = expp8.tile([128, 2, 512], FP8, tag="e8", name="e8")
                    scale = EXPSCALE if j == 0 else EXPSCALE8
                    w0 = cw[0][2]
                    if w0 == 512:       # widths 512, 384
                        sflat = sT[:].rearrange("p a b -> p (a b)")
                        eflat = et[:].rearrange("p a b -> p (a b)")
                        nc.scalar.activation(eflat[:, 0:896], sflat[:, 0:896],
                                             Exp, scale=scale)
                    else:               # widths 256, 128
                        nc.scalar.activation(et[:, :, 0:128], sT[:, :, 0:128],
                                             Exp, scale=scale)
                        nc.scalar.activation(et[:, 0, 128:256], sT[:, 0, 128:256],
                                             Exp, scale=scale)
                    for ci, (c, r, w) in enumerate(cw):
                        if j == 0:
                            nc.vector.tensor_mul(et[:, ci, 0:128],
                                                 et[:, ci, 0:128], mask16_sb[:])
                        else:
                            nc.gpsimd.tensor_mul(et[:, ci, 0:128],
                                                 et[:, ci, 0:128], mask_sb[:])
                    for ci, (c, r, w) in enumerate(cw):
                        if j == 0:
                            v3 = v_sb[c][:].rearrange("p (h d) -> p h d", d=65)
                            lhsT = v3[:, h, :]
                        else:
                            lhsT = v8_sb[c // 2][:, h, c % 2, 0:65]
                        nc.tensor.matmul(oT_ps[:, r:512], lhsT,
                                         et[:, ci, 0:w],
                                         start=(c == 0), stop=(c == nblk - 1),
                                         skip_group_check=True)
                zi = zpool.tile([1, 512], F32, tag="zi", name="zi")
                nc.vector.reciprocal(zi[:], oT_ps[64:65, :])
                zb = zpool.tile([64, 512], F32, tag="zb", name="zb")
                nc.gpsimd.partition_broadcast(zb[:], zi[:])
                nc.vector.tensor_mul(oT_sb[p][j][64 * hh:64 * hh + 64, :],
                                     oT_ps[0:64, :], zb[:])

            def out_proj(j, t):
                ob = outp.tile([128, E], F32, tag="ob", name="ob")
                for n in range(2):
                    ps = ps_mm.tile([128, 512], F32, tag="mm", name="mmo")
                    for p in range(NP):
                        nc.tensor.matmul(
                            ps[:],
                            oT_sb[p][j][:, (t % 4) * 128:(t % 4 + 1) * 128],
                            wpT_all[:, p, n * 512:(n + 1) * 512],
                            start=(p == 0), stop=(p == NP - 1))
                    nc.vector.tensor_copy(ob[:, n * 512:(n + 1) * 512], ps[:])
                    nc.sync.dma_start(
                        out[t * 128:(t + 1) * 128, n * 512:(n + 1) * 512],
                        ob[:, n * 512:(n + 1) * 512])

            # ---- emission schedule ----
            j0_proj(0)
            for i in range(4):
                v_proj(i)

            fillers = {
                0: ([("j0p", 1), ("pq", 1, 0), ("pk", 1, 0), ("j0p", 2),
                     ("j0p", 3), ("pq", 1, 1), ("pk", 1, 1), ("v", 4),
                     ("pq", 1, 2), ("pk", 1, 2), ("v", 5),
                     ("pq", 1, 3), ("pk", 1, 3), ("v", 6), ("v", 7)]),
                1: ([("pq", 2, 0), ("pk", 2, 0), ("v", 8), ("o", 0, 0),
                     ("pq", 2, 1), ("pk", 2, 1), ("v", 9), ("o", 0, 1),
                     ("pq", 2, 2), ("pk", 2, 2), ("v", 10), ("o", 0, 2),
                     ("pq", 2, 3), ("pk", 2, 3), ("v", 11), ("o", 0, 3)]),
                2: ([("pq", 3, 0), ("pk", 3, 0), ("v", 12), ("o", 1, 0),
                     ("pq", 3, 1), ("pk", 3, 1), ("v", 13), ("o", 1, 1),
                     ("pq", 3, 2), ("pk", 3, 2), ("v", 14), ("o", 1, 2),
                     ("pq", 3, 3), ("pk", 3, 3), ("v", 15), ("o", 1, 3)]),
                3: ([("o", 2, 0), ("o", 2, 1), ("o", 2, 2), ("o", 2, 3)]),
            }

            def run_filler(f):
                if f[0] == "v":
                    v_proj(f[1])
                elif f[0] == "j0p":
                    j0_proj(f[1])
                elif f[0] == "pq":
                    jn_proj(f[1], f[2], w8q_all, q8_sb)
                elif f[0] == "pk":
                    jn_proj(f[1], f[2], w8k_all, k8_sb)
                elif f[0] == "o":
                    out_proj(f[1], f[1] * 4 + f[2])

            for j in range(NJ):
                q = list(fillers[j])
                per = (len(q) + HL - 1) // HL
                for h in range(HL):
                    attn(j, h)
                    for _ in range(per):
                        if q:
                            run_filler(q.pop(0))
                while q:
                    run_filler(q.pop(0))
            for t in range(12, 16):
                out_proj(3, t)

    nc.compile()
    return nc


def _mask_np(dt):
    f = np.arange(128)[None, :]
    p = np.arange(128)[:, None]
    return (f >= p).astype(dt)


def kernel(x, Wq, Wk, Wv, Wp, bp):
    x = np.asarray(x, dtype=np.float32)
    Wq = np.asarray(Wq, dtype=np.float32)
    Wk = np.asarray(Wk, dtype=np.float32)
    Wv = np.asarray(Wv, dtype=np.float32)
    Wp = np.asarray(Wp, dtype=np.float32)
    bp = np.asarray(bp, dtype=np.float32)

    if "nc" not in _CACHE:
        _CACHE["nc"] = _build()
    nc = _CACHE["nc"]

    WpT = np.ascontiguousarray(Wp.T).astype(bfnp)
    xTs = [np.ascontiguousarray(x[b].T) for b in range(B)]

    def wslice(W, h0, scale, dt):
        return np.ascontiguousarray(
            (W[h0:h0 + HL] * scale).transpose(1, 0, 2).reshape(E, HL * D)).astype(dt)

    in_maps = []
    for c in range(8):
        b, hhf = c // 2, c % 2
        h0 = hhf * HL
        in_maps.append({
            "xT": xTs[b].astype(bfnp),
            "xT8": xTs[b].astype(fp8np),
            "wq": wslice(Wq, h0, 1.0, bfnp),
            "wk": wslice(Wk, h0, 1.0, bfnp),
            "wv": wslice(Wv, h0, 1.0, bfnp),
            "w8q": wslice(Wq, h0, WSCALE, fp8np),
            "w8k": wslice(Wk, h0, WSCALE, fp8np),
            "w8v": wslice(Wv, h0, VSCALE, fp8np),
            "wpT": np.ascontiguousarray(WpT[h0 * D:(h0 + HL) * D, :]),
            "maskb": _mask_np(fp8np),
            "maskb16": _mask_np(bfnp),
        })

    res = run_bass_kernel_spmd(nc, in_maps, list(range(8)))
    parts = [np.asarray(res.results[c]["out"], dtype=np.float32) for c in range(8)]
    out = np.stack([parts[2 * b] + parts[2 * b + 1] for b in range(B)], axis=0)
    return (out + bp[None, None, :]).astype(np.float32)
